# revision 32
# baseline (speedup 1.0000x reference)
"""Fused transformer block (QKV proj + attention + FFN + 2x LayerNorm) on 8
Trainium2 NeuronCores.

Sharding: batch (B=2) across two 4-core groups; within a group, tensor
parallel over heads (4 heads / core) for projections+attention, then an
AllToAll switches to row (sequence) sharding for the FFN/LayerNorm tail.

v2 design notes (vs the f32r baseline):
- Host pre-transposes Q/K and pre-packs every tensor partition-major in
  bf16, so there are no on-device input transposes and every DMA line is
  partition-contiguous.
- The AllGather of Kp^T is gone: Vp = Kp@Wv = K@(Wk@Wv) + (bk@Wv + bv),
  with Wkv fused on device from a host-supplied Wk^T (layout-only prep).
- All big matmuls run in bf16 (1 cycle/row on the PE vs 2 for f32r).
- attn@V runs in fp8e4 with DoubleRow perf mode (2 rows/cycle), with the
  softmax denominator fused in as a ones-column of V.
- exp() is split across the scalar (Act) engine and the DVE/GpSimd
  engines; the latter two use a Schraudolph bit-trick exp (~3% rel err,
  harmless under softmax) since only the Act engine has native Exp.
- Softmax normalization + Q residual happen in natural layout after a
  PE transpose of the PSUM attention output, killing the [1,512]
  reciprocals and partition broadcasts of the baseline.
"""
import sys

import numpy as np

try:
    import concourse.bass  # noqa: F401
except ImportError:
    sys.path.insert(0, "/opt/trn_rl_repo")

import ml_dtypes

import concourse.bacc as bacc
import concourse.mybir as mybir
import concourse.tile as tile
from concourse import bass_utils
from concourse.masks import make_identity

P = 128
S = 2048          # sequence length (Sq == Sk)
D = 1024          # model dim
H = 16            # total heads
DH = 64           # head dim
NCORES = 8
GROUP = 4         # cores per batch group
JC = D // GROUP   # 256 local projection columns
HL = JC // DH     # 4 local heads
DCH = D // P      # 8 d chunks
SCH = S // P      # 16 s chunks
QB = 512          # q block for attention
NQB = S // QB     # 4
SR2 = 2 * S // NCORES  # 512 output rows per core (256 per batch)

F32 = mybir.dt.float32
BF16 = mybir.dt.bfloat16
FP8 = mybir.dt.float8e4
I32 = mybir.dt.int32
AF = mybir.ActivationFunctionType
OP = mybir.AluOpType
DR = mybir.MatmulPerfMode.DoubleRow
EPS = 1e-5

# Schraudolph fast-exp constants: exp(y) ~= bitcast_f32(i32(y*EXA + EXB))
# calibrated for truncation, max rel err ~3.0% over y in [-14, 6].
EXA = 12102203.161561485        # 2^23 / ln(2)
EXB = float((127 << 23) - 366400)
# softmax shift: exp(s*0.125 - SM_SHIFT) keeps e well under the fp8e4 max of
# 240 (values >= ~272 become inf) for rows with large ||q||; softmax is
# invariant to the shift since the ones-column denominator scales equally.
SM_SHIFT = 4.5
# exp engine split per g-chunk of each attention unit (8 chunks of
# [128,1024] exps). Only the Act engine (native Exp) and the DVE
# (Schraudolph bit-trick, ~1.8us/chunk incl. the fp8 cast) can read PSUM;
# alternate 3/2 DVE chunks per unit to balance ~6.1us/unit on each engine.
DVE_CHUNKS = (2, 2)

NPBF16 = ml_dtypes.bfloat16

# host-side q permutation for the chunked AllToAll: s' = i*512 + m*64 + j
# maps to original row s = m*256 + i*64 + j (i = dest row-slice, m = dest
# core within batch group). The unshard mapping is unchanged.
_PERM = np.array([m * 256 + i * 64 + j
                  for i in range(4) for m in range(8) for j in range(64)])

_CACHE: dict = {}


def _declare_io(nc):
    t = {}
    t["qt"] = nc.dram_tensor("qt", [P, DCH * S], BF16, kind="ExternalInput").ap()
    t["kt"] = nc.dram_tensor("kt", [P, DCH * S], BF16, kind="ExternalInput").ap()
    for w in ("wq", "wk", "wv"):
        t[w] = nc.dram_tensor(w, [P, DCH * JC], BF16, kind="ExternalInput").ap()
    t["wkt"] = nc.dram_tensor("wkt", [P, DCH * D], BF16, kind="ExternalInput").ap()
    t["wo"] = nc.dram_tensor("wo", [P, DCH * D], BF16, kind="ExternalInput").ap()
    t["bq_h"] = nc.dram_tensor("bq_h", [DH, HL], F32, kind="ExternalInput").ap()
    t["bk_h"] = nc.dram_tensor("bk_h", [DH, HL], F32, kind="ExternalInput").ap()
    t["bkc"] = nc.dram_tensor("bkc", [P, DCH], BF16, kind="ExternalInput").ap()
    t["bv_s"] = nc.dram_tensor("bv_s", [1, JC], F32, kind="ExternalInput").ap()
    for b in ("bo", "g0", "b0", "g1", "b1"):
        t[b] = nc.dram_tensor(b, [1, D], F32, kind="ExternalInput").ap()
    t["out"] = nc.dram_tensor("out", [SR2, D], F32, kind="ExternalOutput").ap()
    return t


def _emit(nc, tc, ctx, t):
    # ---- psum pools: ps_s 2x4KB + (psA 2x2KB, pst 2x2KB) = 16KB ----
    ps_s = ctx.enter_context(tc.tile_pool(name="ps_s", bufs=2, space="PSUM"))
    ps_a = ctx.enter_context(tc.tile_pool(name="ps_a", bufs=2, space="PSUM"))
    dram = ctx.enter_context(tc.tile_pool(name="dram", bufs=1, space="DRAM"))

    const = ctx.enter_context(tc.tile_pool(name="const", bufs=1))

    # ---- constants / small params ----
    ident_f = const.tile([P, P], F32)
    make_identity(nc, ident_f)
    ident_b = const.tile([P, P], BF16)
    nc.vector.tensor_copy(ident_b[:], ident_f[:])
    eps_t = const.tile([P, 1], F32)
    nc.vector.memset(eps_t, EPS)
    smshift_t = const.tile([P, 1], F32)
    nc.vector.memset(smshift_t, -SM_SHIFT)

    bq_sb = const.tile([DH, HL], F32)
    nc.sync.dma_start(bq_sb[:], t["bq_h"])
    bk_sb = const.tile([DH, HL], F32)
    nc.sync.dma_start(bk_sb[:], t["bk_h"])
    bkc_sb = const.tile([P, DCH], BF16)
    nc.sync.dma_start(bkc_sb[:], t["bkc"])
    bv_sb = const.tile([1, JC], F32)
    nc.sync.dma_start(bv_sb[:], t["bv_s"])

    bcast = {}
    for b in ("bo", "g0", "b0", "g1", "b1"):
        bcast[b] = const.tile([P, D], F32, name=f"bcast_{b}")
        nc.gpsimd.dma_start(bcast[b][:], t[b].to_broadcast([P, D]))

    # ---- persistent activations ----
    heads_cm = tc.tile_pool(name="heads", bufs=1)
    heads = heads_cm.__enter__()
    q_heads = heads.tile([DH, HL, S], BF16)
    k_heads = heads.tile([DH, HL, S], BF16)
    # V padded to 80 cols: 0:64 = V, 64 = ones (softmax denominator), 65:80
    # zero pad: the DoubleRow ldweights AP requires the k-tile stride to be
    # 16-byte aligned. k-tile pairs are adjacent (dim 3).
    vp = heads.tile([P, SCH // 2, HL, 2, DH + 16], FP8)
    oh_nat = heads.tile([P, SCH, JC], BF16)

    # ---- weights + transposed inputs (freed before attention) ----
    wx_cm = tc.tile_pool(name="wx", bufs=1)
    wx = wx_cm.__enter__()
    wkt_sb = wx.tile([P, DCH, D], BF16)
    for ec in range(DCH):
        nc.sync.dma_start(
            wkt_sb[:, ec, :],
            t["wkt"].rearrange("p (c n) -> p c n", c=DCH)[:, ec, :])
    wv_sb = wx.tile([P, DCH, JC], BF16)
    nc.sync.dma_start(wv_sb[:], t["wv"].rearrange("p (c n) -> p c n", c=DCH))
    wk_sb = wx.tile([P, DCH, JC], BF16)
    nc.sync.dma_start(wk_sb[:], t["wk"].rearrange("p (c n) -> p c n", c=DCH))
    wq_sb = wx.tile([P, DCH, JC], BF16)
    nc.sync.dma_start(wq_sb[:], t["wq"].rearrange("p (c n) -> p c n", c=DCH))
    wkv_sb = wx.tile([P, DCH, JC], BF16)

    x_cm = tc.tile_pool(name="x", bufs=1)
    xp = x_cm.__enter__()
    kt_sb = xp.tile([P, DCH, S], BF16)
    for dc in range(DCH):
        nc.sync.dma_start(
            kt_sb[:, dc, :],
            t["kt"].rearrange("p (c n) -> p c n", c=DCH)[:, dc, :])
    qt_sb = xp.tile([P, DCH, S], BF16)
    for dc in range(DCH):
        nc.sync.dma_start(
            qt_sb[:, dc, :],
            t["qt"].rearrange("p (c n) -> p c n", c=DCH)[:, dc, :])
    wo_sb = const.tile([P, DCH, D], BF16)
    nc.sync.dma_start(wo_sb[:], t["wo"].rearrange("p (c n) -> p c n", c=DCH))

    # ---- Wkv = Wk @ Wv (local JC columns); vp bias = bk @ Wv + bv ----
    for dc in range(DCH):
        psW = ps_s.tile([P, JC], F32, tag="ps_s")
        for ec in range(DCH):
            nc.tensor.matmul(psW[:], wkt_sb[:, ec, dc * P:(dc + 1) * P],
                             wv_sb[:, ec, :], start=(ec == 0), stop=(ec == DCH - 1))
        nc.vector.tensor_copy(wkv_sb[:, dc, :], psW[:])
    psB = ps_s.tile([1, JC], F32, tag="ps_s")
    for ec in range(DCH):
        nc.tensor.matmul(psB[:], bkc_sb[:, ec:ec + 1], wv_sb[:, ec, :],
                         start=(ec == 0), stop=(ec == DCH - 1))
    vpb_row = const.tile([1, JC], F32)
    nc.vector.tensor_tensor(out=vpb_row[:], in0=psB[:], in1=bv_sb[:], op=OP.add)
    vpb = const.tile([P, JC], F32)
    nc.gpsimd.partition_broadcast(vpb[:], vpb_row[:], channels=P)

    # ---- Kp^T / Qp^T projections: heads[j, s] = sum_d W[d, j] X^T[d, s] ----
    # Bias adds split between the Act engine (K path) and the DVE (Q path)
    # so the prolog is not DVE-serialized.
    def project(w_sb, x_sb, bias_sb, dst):
        for jc2 in range(JC // P):          # 2 head-pairs
            for nb in range(S // QB):       # 4 s-blocks
                ps = ps_s.tile([P, QB], F32, tag="ps_s")
                for dc in range(DCH):
                    nc.tensor.matmul(
                        ps[:], w_sb[:, dc, jc2 * P:(jc2 + 1) * P],
                        x_sb[:, dc, nb * QB:(nb + 1) * QB],
                        start=(dc == 0), stop=(dc == DCH - 1))
                for hh in range(2):
                    h = 2 * jc2 + hh
                    nc.vector.tensor_scalar(
                        out=dst[:, h, nb * QB:(nb + 1) * QB],
                        in0=ps[hh * DH:(hh + 1) * DH, :],
                        scalar1=bias_sb[:, h:h + 1], scalar2=None,
                        op0=OP.add)
    project(wk_sb, kt_sb, bk_sb, k_heads)

    # ---- Vp natural [s, j] = sum_d K^T[d, s]^T Wkv[d, j], + bias, fp8 ----
    for sc in range(SCH):
        psV = ps_s.tile([P, JC], F32, tag="ps_s")
        for dc in range(DCH):
            nc.tensor.matmul(psV[:], kt_sb[:, dc, sc * P:(sc + 1) * P],
                             wkv_sb[:, dc, :], start=(dc == 0), stop=(dc == DCH - 1))
        g2, i2 = divmod(sc, 2)
        nc.vector.tensor_tensor(
            out=vp[:, g2, :, i2, 0:DH],
            in0=psV.rearrange("p (h d) -> p h d", h=HL),
            in1=vpb.rearrange("p (h d) -> p h d", h=HL), op=OP.add)
    nc.vector.memset(vp[:, :, :, :, DH:DH + 1], 1.0)
    nc.vector.memset(vp[:, :, :, :, DH + 1:DH + 16], 0.0)

    project(wq_sb, qt_sb, bq_sb, q_heads)

    x_cm.__exit__(None, None, None)        # free kt/qt (64 KB/part)
    wx_cm.__exit__(None, None, None)       # free weights (32 KB/part)

    # ---- attention: software-pipelined chunk stream ----
    # Chunks (unit, g) run in a flat stream; attn@V lags SKEW chunks behind
    # the scores so the in-order PE queue never blocks on an exp still in
    # flight. Each unit's normalize/transpose tail is deferred into the next
    # unit's stream slots the same way.
    from collections import deque

    epool_cm = tc.tile_pool(name="epool", bufs=7)
    epool = epool_cm.__enter__()
    ipool_cm = tc.tile_pool(name="ipool", bufs=4)
    ipool = ipool_cm.__enter__()
    opool_cm = tc.tile_pool(name="opool", bufs=2)
    opool = opool_cm.__enter__()

    # chunked AllToAll: the host permutes Q's sequence dim so q-block qb
    # holds exactly the rows destined to row-slice qb of every core; each
    # 256KB collective fires as soon as its q-block's attention completes
    # and overlaps the remaining attention.
    # each slice is further split into two column-half collectives (heads
    # 0-1 / heads 2-3) so the second fires two units earlier and the last
    # exposed collective is only 128KB.
    a2a_in = [[dram.tile([QB, JC // 2], BF16, name=f"a2a_in{i}_{hf}")
               for hf in range(2)] for i in range(NQB)]
    a2a_out = [[dram.tile([QB, JC // 2], BF16, name=f"a2a_out{i}_{hf}")
                for hf in range(2)] for i in range(NQB)]

    NG = SCH // 2                           # 8 kc-pair groups
    UNITS = [(qb, h) for qb in range(NQB) for h in range(HL)]
    NCH = len(UNITS) * NG
    SKEW = 5

    e_tiles = {}
    psA_tiles = {}
    pending = deque()

    def emit_scores(c):
        u, g = divmod(c, NG)
        qb, h = UNITS[u]
        qsl = slice(qb * QB, (qb + 1) * QB)
        psS = ps_s.tile([P, 2 * QB], F32, tag="ps_s")
        for i in range(2):
            kc = 2 * g + i
            nc.tensor.matmul(
                psS[:, i * QB:(i + 1) * QB],
                k_heads[:, h, kc * P:(kc + 1) * P],
                q_heads[:, h, qsl], start=True, stop=True)
        e_sb = epool.tile([P, 2, QB], FP8, tag="e")
        if g >= DVE_CHUNKS[u % 2]:
            nc.scalar.activation(
                e_sb.rearrange("p a b -> p (a b)"), psS[:], AF.Exp,
                scale=0.125, bias=smshift_t[:])
        else:
            i32_sb = ipool.tile([P, 2 * QB], I32, tag="i32")
            nc.vector.tensor_scalar(
                out=i32_sb[:], in0=psS[:], scalar1=EXA * 0.125,
                scalar2=EXB - SM_SHIFT * EXA, op0=OP.mult, op1=OP.add)
            nc.vector.tensor_copy(
                e_sb.rearrange("p a b -> p (a b)"), i32_sb.bitcast(F32))
        e_tiles[c] = e_sb

    def emit_pv(c):
        u, g = divmod(c, NG)
        qb, h = UNITS[u]
        if g == 0:
            psA_tiles[u] = ps_a.tile([DH + 16, QB], F32, tag="psA",
                                     name="psA")
        nc.tensor.matmul(
            psA_tiles[u][:], vp[:, g, h, :, :], e_tiles.pop(c)[:],
            start=(g == 0), stop=(g == NG - 1), perf_mode=DR)
        if g == NG - 1:
            pending.extend(_post_pieces(u))

    def _post_pieces(u):
        qb, h = UNITS[u]
        psA = psA_tiles.pop(u)
        st = {}

        def p_copy():
            st["oht"] = opool.tile([DH + 1, QB], BF16, name="oht")
            nc.vector.tensor_copy(st["oht"][:], psA[0:DH + 1, :])

        def p_tr(qc0):
            def f():
                if "pst" not in st:
                    st["pst"] = ps_a.tile([P, NQB, 2 * P], BF16,
                                          tag="pst", name="pst")
                oht, pst = st["oht"], st["pst"]
                for qc in (qc0, qc0 + 1):
                    nc.tensor.transpose(
                        pst[:, qc, 0:DH + 1],
                        oht[:, qc * P:(qc + 1) * P],
                        ident_b[0:DH + 1, 0:DH + 1])
                    nc.tensor.transpose(
                        pst[:, qc, DH + 2:2 * DH + 2],
                        q_heads[:, h,
                                (qb * NQB + qc) * P:(qb * NQB + qc + 1) * P],
                        ident_b[0:DH, 0:DH])
            return f

        def p_norm():
            pst = st["pst"]
            rec = opool.tile([P, NQB, 1], F32, name="rec")
            nc.vector.reciprocal(rec[:], pst[:, :, DH:DH + 1])
            for qc in range(NQB):
                sc = qb * NQB + qc
                nc.vector.tensor_scalar(
                    out=oh_nat[:, sc, h * DH:(h + 1) * DH],
                    in0=pst[:, qc, 0:DH], scalar1=rec[:, qc, :],
                    scalar2=None, op0=OP.mult)

        def p_res():
            pst = st["pst"]
            nc.vector.tensor_tensor(
                out=oh_nat[:, qb * NQB:(qb + 1) * NQB, h * DH:(h + 1) * DH],
                in0=oh_nat[:, qb * NQB:(qb + 1) * NQB, h * DH:(h + 1) * DH],
                in1=pst[:, :, DH + 2:2 * DH + 2], op=OP.add)

        pieces = [p_copy, None, None, p_tr(0), p_tr(2), p_norm, p_res]
        if h % 2 == 1:
            hf = h // 2
            def p_dma():
                for qc in range(NQB):
                    sc = qb * NQB + qc
                    nc.sync.dma_start(
                        a2a_in[qb][hf][qc * P:(qc + 1) * P, :],
                        oh_nat[:, sc, hf * (JC // 2):(hf + 1) * (JC // 2)])
                nc.gpsimd.collective_compute(
                    "AllToAll", OP.bypass, ins=[a2a_in[qb][hf].opt()],
                    outs=[a2a_out[qb][hf].opt()],
                    replica_groups=[list(range(NCORES))])
            pieces.append(p_dma)
        return pieces

    for c in range(NCH + SKEW):
        if c >= SKEW:
            emit_pv(c - SKEW)
        if c < NCH:
            emit_scores(c)
        if pending:
            piece = pending.popleft()
            if piece is not None:
                piece()
    while pending:
        piece = pending.popleft()
        if piece is not None:
            piece()

    opool_cm.__exit__(None, None, None)
    ipool_cm.__exit__(None, None, None)
    epool_cm.__exit__(None, None, None)
    heads_cm.__exit__(None, None, None)

    # ---- stage 2: rows [SR2, D] : LN0 -> Wo+gelu+residual -> LN1 ----
    s2 = ctx.enter_context(tc.tile_pool(name="s2", bufs=1))
    ln_tmp = ctx.enter_context(tc.tile_pool(name="ln_tmp", bufs=4))
    NS2 = SR2 // P                                    # 4 row chunks
    # chunked-A2A gather: a2a_out[qb] rows [64c:64c+64) hold, for peer c,
    # this core's local rows 256*(c//4) + 64*qb + [0,64) (batch c//4),
    # columns [256*(c%4):...). Local row r -> o_sb[r%128, r//128, :].
    o_sb = s2.tile([P, NS2, D], BF16)
    for qb in range(NQB):
        for c in range(NCORES):
            bb, jj = divmod(c, GROUP)
            rloc = 256 * bb + 64 * qb
            sc2, rr = divmod(rloc, P)
            for hf in range(2):
                nc.sync.dma_start(
                    o_sb[rr:rr + 64, sc2,
                         jj * JC + hf * (JC // 2):
                         jj * JC + (hf + 1) * (JC // 2)],
                    a2a_out[qb][hf][c * 64:(c + 1) * 64, :])

    def layernorm(src_ap, dst_ap, gb, bb):
        """src [128, D] -> dst [128, D] layernorm with broadcast gamma/beta."""
        red = ln_tmp.tile([P, 1], F32, tag="red")
        nc.vector.tensor_reduce(red[:], src_ap, mybir.AxisListType.X, OP.add)
        negmean = ln_tmp.tile([P, 1], F32, tag="negmean")
        nc.vector.tensor_scalar_mul(negmean[:], red[:], -1.0 / D)
        sq = ln_tmp.tile([P, D], BF16, tag="sq")
        sumsq = ln_tmp.tile([P, 1], F32, tag="sumsq")
        nc.scalar.activation(sq[:], src_ap, AF.Square, bias=negmean[:],
                             scale=1.0, accum_out=sumsq[:])
        std = ln_tmp.tile([P, 1], F32, tag="std")
        nc.scalar.activation(std[:], sumsq[:], AF.Sqrt, bias=eps_t[:],
                             scale=1.0 / D)
        rstd = ln_tmp.tile([P, 1], F32, tag="rstd")
        nc.vector.reciprocal(rstd[:], std[:])
        nc.vector.tensor_scalar(out=dst_ap, in0=src_ap, scalar1=negmean[:],
                                scalar2=rstd[:], op0=OP.add, op1=OP.mult)
        nc.vector.tensor_tensor(out=dst_ap, in0=dst_ap, in1=gb[:], op=OP.mult)
        nc.vector.tensor_tensor(out=dst_ap, in0=dst_ap, in1=bb[:], op=OP.add)

    # sc2 chunks 0 and 2 only need the first two A2A chunks - process them
    # first so LN0 overlaps the tail of attention.
    SC2_ORDER = (0, 2, 1, 3)
    ln0 = s2.tile([P, NS2, D], BF16)
    for sc2 in SC2_ORDER:
        layernorm(o_sb[:, sc2, :], ln0[:, sc2, :], bcast["g0"], bcast["b0"])

    # transpose ln0 -> [128, DCH, SR2] for the Wo contraction
    ln0t = s2.tile([P, DCH, SR2], BF16)
    for dc in range(DCH):
        psL = ps_a.tile([P, SR2], BF16, tag="pst")
        for sc2 in range(NS2):
            nc.tensor.transpose(psL[:, sc2 * P:(sc2 + 1) * P],
                                ln0[:, sc2, dc * P:(dc + 1) * P], ident_b)
        nc.vector.tensor_copy(ln0t[:, dc, :], psL[:])

    o2 = s2.tile([P, NS2, D], BF16)
    for sc2 in SC2_ORDER:
        psF = ps_s.tile([P, D], F32, tag="ps_s")
        for dc in range(DCH):
            for nb in range(2):
                nc.tensor.matmul(
                    psF[:, nb * QB:(nb + 1) * QB],
                    ln0t[:, dc, sc2 * P:(sc2 + 1) * P],
                    wo_sb[:, dc, nb * QB:(nb + 1) * QB],
                    start=(dc == 0), stop=(dc == DCH - 1))
        fb = ln_tmp.tile([P, D], F32, tag="fb")
        nc.vector.tensor_tensor(out=fb[:], in0=psF[:], in1=bcast["bo"][:],
                                op=OP.add)
        gel = ln_tmp.tile([P, D], BF16, tag="gel")
        nc.scalar.activation(gel[:], fb[:], AF.Gelu)
        nc.vector.tensor_tensor(out=o2[:, sc2, :], in0=ln0[:, sc2, :],
                                in1=gel[:], op=OP.add)

    for sc2 in SC2_ORDER:
        fin = ln_tmp.tile([P, D], F32, tag="fin")
        layernorm(o2[:, sc2, :], fin[:], bcast["g1"], bcast["b1"])
        nc.sync.dma_start(t["out"][sc2 * P:(sc2 + 1) * P, :], fin[:])


def build():
    if "nc" in _CACHE:
        return _CACHE["nc"]
    from contextlib import ExitStack
    nc = bacc.Bacc("TRN2", target_bir_lowering=False, debug=False,
                   num_devices=NCORES)
    t = _declare_io(nc)
    with tile.TileContext(nc) as tc:
        with ExitStack() as ctx:
            _emit(nc, tc, ctx, t)
    nc.compile()
    _CACHE["nc"] = nc
    return nc


def _pmajor(a):
    """[D0, N] with D0 = c*128+p  ->  [128, c*N] partition-major bf16."""
    d0, n = a.shape
    c = d0 // P
    return np.ascontiguousarray(
        a.reshape(c, P, n).transpose(1, 0, 2).reshape(P, c * n).astype(NPBF16))


def make_in_maps(Q, K, Wq, bq, Wk, bk, Wv, bv, Wo, bo, g0, b0, g1, b1):
    f32 = np.float32
    in_maps = []
    wkt_full = _pmajor(np.ascontiguousarray(Wk.T))
    wo_full = _pmajor(Wo)
    bkc = np.ascontiguousarray(
        bk.reshape(DCH, P).T.astype(NPBF16))          # [128, DCH]
    qt = {}
    kt = {}
    for b in range(2):
        qt[b] = _pmajor(np.ascontiguousarray(Q[b].T[:, _PERM]))
        kt[b] = _pmajor(np.ascontiguousarray(K[b].T))
    for c in range(NCORES):
        b, g = divmod(c, GROUP)
        jsl = slice(g * JC, (g + 1) * JC)
        in_maps.append({
            "qt": qt[b], "kt": kt[b],
            "wq": _pmajor(Wq[:, jsl]), "wk": _pmajor(Wk[:, jsl]),
            "wv": _pmajor(Wv[:, jsl]),
            "wkt": wkt_full, "wo": wo_full,
            "bq_h": np.ascontiguousarray(
                bq[jsl].reshape(HL, DH).T.astype(f32)),
            "bk_h": np.ascontiguousarray(
                bk[jsl].reshape(HL, DH).T.astype(f32)),
            "bkc": bkc,
            "bv_s": np.ascontiguousarray(bv[jsl].reshape(1, JC).astype(f32)),
            "bo": np.ascontiguousarray(bo.reshape(1, D).astype(f32)),
            "g0": np.ascontiguousarray(g0.reshape(1, D).astype(f32)),
            "b0": np.ascontiguousarray(b0.reshape(1, D).astype(f32)),
            "g1": np.ascontiguousarray(g1.reshape(1, D).astype(f32)),
            "b1": np.ascontiguousarray(b1.reshape(1, D).astype(f32)),
        })
    return in_maps


def run(in_maps, trace=False, **kwargs):
    nc = build()
    return bass_utils.run_bass_kernel_spmd(
        nc, in_maps, core_ids=list(range(NCORES)), trace=trace, **kwargs)


def kernel(**inputs):
    inputs = {k: np.asarray(v, dtype=np.float32) for k, v in inputs.items()}
    in_maps = make_in_maps(
        inputs["Q"], inputs["K"], inputs["Wq"], inputs["bq"], inputs["Wk"],
        inputs["bk"], inputs["Wv"], inputs["bv"], inputs["Wo"], inputs["bo"],
        inputs["g0"], inputs["b0"], inputs["g1"], inputs["b1"])
    res = run(in_maps, trace=False)
    B = 2
    RS = S // NCORES  # 256 rows of each batch per core
    out = np.empty((B, S, D), dtype=np.float32)
    for c in range(NCORES):
        r = res.results[c]["out"]  # [512, D]: rows 0-255 -> b0, 256-511 -> b1
        out[0, c * RS:(c + 1) * RS, :] = r[:RS]
        out[1, c * RS:(c + 1) * RS, :] = r[RS:]
    return out


if __name__ == "__main__":
    rng = np.random.default_rng(0)
    ins = {n: rng.standard_normal(s).astype(np.float32) * (0.03125 if n.startswith("W") else 1.0)
           for n, s in [("Q", (2, S, D)), ("K", (2, S, D)), ("Wq", (D, D)),
                        ("Wk", (D, D)), ("Wv", (D, D)), ("Wo", (D, D))]}
    for n in ("bq", "bk", "bv", "bo", "b0", "b1"):
        ins[n] = np.zeros(D, np.float32)
    for n in ("g0", "g1"):
        ins[n] = np.ones(D, np.float32)
    out = kernel(**ins)
    print("ran ok", out.shape, out.dtype)


# revision 34
# speedup vs baseline: 1.0839x; 1.0839x over previous
"""Fused transformer block (QKV proj + attention + FFN + 2x LayerNorm) on 8
Trainium2 NeuronCores.

Sharding: batch (B=2) across two 4-core groups; within a group, tensor
parallel over heads (4 heads / core) for projections+attention, then an
AllToAll switches to row (sequence) sharding for the FFN/LayerNorm tail.

v2 design notes (vs the f32r baseline):
- Host pre-transposes Q/K and pre-packs every tensor partition-major in
  bf16, so there are no on-device input transposes and every DMA line is
  partition-contiguous.
- The AllGather of Kp^T is gone: Vp = Kp@Wv = K@(Wk@Wv) + (bk@Wv + bv),
  with Wkv fused on device from a host-supplied Wk^T (layout-only prep).
- All big matmuls run in bf16 (1 cycle/row on the PE vs 2 for f32r).
- attn@V runs in fp8e4 with DoubleRow perf mode (2 rows/cycle), with the
  softmax denominator fused in as a ones-column of V.
- exp() is split across the scalar (Act) engine and the DVE/GpSimd
  engines; the latter two use a Schraudolph bit-trick exp (~3% rel err,
  harmless under softmax) since only the Act engine has native Exp.
- Softmax normalization + Q residual happen in natural layout after a
  PE transpose of the PSUM attention output, killing the [1,512]
  reciprocals and partition broadcasts of the baseline.
"""
import sys

import numpy as np

try:
    import concourse.bass  # noqa: F401
except ImportError:
    sys.path.insert(0, "/opt/trn_rl_repo")

import ml_dtypes

import concourse.bacc as bacc
import concourse.mybir as mybir
import concourse.tile as tile
from concourse import bass_utils
from concourse.masks import make_identity

P = 128
S = 2048          # sequence length (Sq == Sk)
D = 1024          # model dim
H = 16            # total heads
DH = 64           # head dim
NCORES = 8
GROUP = 4         # cores per batch group
JC = D // GROUP   # 256 local projection columns
HL = JC // DH     # 4 local heads
DCH = D // P      # 8 d chunks
SCH = S // P      # 16 s chunks
QB = 512          # q block for attention
NQB = S // QB     # 4
SR2 = 2 * S // NCORES  # 512 output rows per core (256 per batch)

F32 = mybir.dt.float32
BF16 = mybir.dt.bfloat16
FP8 = mybir.dt.float8e4
I32 = mybir.dt.int32
AF = mybir.ActivationFunctionType
OP = mybir.AluOpType
DR = mybir.MatmulPerfMode.DoubleRow
EPS = 1e-5

# Schraudolph fast-exp constants: exp(y) ~= bitcast_f32(i32(y*EXA + EXB))
# calibrated for truncation, max rel err ~3.0% over y in [-14, 6].
EXA = 12102203.161561485        # 2^23 / ln(2)
EXB = float((127 << 23) - 366400)
# softmax shift: exp(s*0.125 - SM_SHIFT) keeps e well under the fp8e4 max of
# 240 (values >= ~272 become inf) for rows with large ||q||; softmax is
# invariant to the shift since the ones-column denominator scales equally.
SM_SHIFT = 4.5
# exp engine split per g-chunk of each attention unit (8 chunks of
# [128,1024] exps). Only the Act engine (native Exp) and the DVE
# (Schraudolph bit-trick, ~1.8us/chunk incl. the fp8 cast) can read PSUM;
# alternate 3/2 DVE chunks per unit to balance ~6.1us/unit on each engine.
DVE_CHUNKS = (2, 2)

NPBF16 = ml_dtypes.bfloat16

# host-side q permutation for the chunked AllToAll: s' = i*512 + m*64 + j
# maps to original row s = m*256 + i*64 + j (i = dest row-slice, m = dest
# core within batch group). The unshard mapping is unchanged.
_PERM = np.array([m * 256 + i * 64 + j
                  for i in range(4) for m in range(8) for j in range(64)])

_CACHE: dict = {}


def _declare_io(nc):
    t = {}
    t["qt"] = nc.dram_tensor("qt", [P, DCH * S], BF16, kind="ExternalInput").ap()
    t["kt"] = nc.dram_tensor("kt", [P, DCH * S], BF16, kind="ExternalInput").ap()
    for w in ("wq", "wk", "wv"):
        t[w] = nc.dram_tensor(w, [P, DCH * JC], BF16, kind="ExternalInput").ap()
    t["wkt"] = nc.dram_tensor("wkt", [P, DCH * D], BF16, kind="ExternalInput").ap()
    t["wo"] = nc.dram_tensor("wo", [P, DCH * D], BF16, kind="ExternalInput").ap()
    t["bq_h"] = nc.dram_tensor("bq_h", [DH, HL], F32, kind="ExternalInput").ap()
    t["bk_h"] = nc.dram_tensor("bk_h", [DH, HL], F32, kind="ExternalInput").ap()
    t["bkc"] = nc.dram_tensor("bkc", [P, DCH], BF16, kind="ExternalInput").ap()
    t["bv_s"] = nc.dram_tensor("bv_s", [1, JC], F32, kind="ExternalInput").ap()
    for b in ("bo", "g0", "b0", "g1", "b1"):
        t[b] = nc.dram_tensor(b, [1, D], F32, kind="ExternalInput").ap()
    t["out"] = nc.dram_tensor("out", [SR2, D], F32, kind="ExternalOutput").ap()
    return t


def _emit(nc, tc, ctx, t):
    # ---- psum pools: ps_s 2x4KB + (psA 2x2KB, pst 2x2KB) = 16KB ----
    ps_s = ctx.enter_context(tc.tile_pool(name="ps_s", bufs=2, space="PSUM"))
    ps_a = ctx.enter_context(tc.tile_pool(name="ps_a", bufs=2, space="PSUM"))
    dram = ctx.enter_context(tc.tile_pool(name="dram", bufs=1, space="DRAM"))

    const = ctx.enter_context(tc.tile_pool(name="const", bufs=1))

    # ---- constants / small params ----
    ident_f = const.tile([P, P], F32)
    make_identity(nc, ident_f)
    ident_b = const.tile([P, P], BF16)
    nc.vector.tensor_copy(ident_b[:], ident_f[:])
    eps_t = const.tile([P, 1], F32)
    nc.vector.memset(eps_t, EPS)
    smshift_t = const.tile([P, 1], F32)
    nc.vector.memset(smshift_t, -SM_SHIFT)

    bq_sb = const.tile([DH, HL], F32)
    nc.sync.dma_start(bq_sb[:], t["bq_h"])
    bk_sb = const.tile([DH, HL], F32)
    nc.sync.dma_start(bk_sb[:], t["bk_h"])
    bkc_sb = const.tile([P, DCH], BF16)
    nc.sync.dma_start(bkc_sb[:], t["bkc"])
    bv_sb = const.tile([1, JC], F32)
    nc.sync.dma_start(bv_sb[:], t["bv_s"])

    bcast = {}
    for b in ("bo", "g0", "b0", "g1", "b1"):
        bcast[b] = const.tile([P, D], F32, name=f"bcast_{b}")
        nc.gpsimd.dma_start(bcast[b][:], t[b].to_broadcast([P, D]))

    # ---- persistent activations ----
    heads_cm = tc.tile_pool(name="heads", bufs=1)
    heads = heads_cm.__enter__()
    q_heads = heads.tile([DH, HL, S], BF16)
    k_heads = heads.tile([DH, HL, S], BF16)
    # V padded to 80 cols: 0:64 = V, 64 = ones (softmax denominator), 65:80
    # zero pad: the DoubleRow ldweights AP requires the k-tile stride to be
    # 16-byte aligned. k-tile pairs are adjacent (dim 3).
    vp = heads.tile([P, SCH // 2, HL, 2, DH + 16], FP8)
    oh_nat = heads.tile([P, SCH, JC], BF16)

    # ---- weights + transposed inputs (freed before attention) ----
    wx_cm = tc.tile_pool(name="wx", bufs=1)
    wx = wx_cm.__enter__()
    wkt_sb = wx.tile([P, DCH, D], BF16)
    for ec in range(DCH):
        nc.sync.dma_start(
            wkt_sb[:, ec, :],
            t["wkt"].rearrange("p (c n) -> p c n", c=DCH)[:, ec, :])
    wv_sb = wx.tile([P, DCH, JC], BF16)
    nc.sync.dma_start(wv_sb[:], t["wv"].rearrange("p (c n) -> p c n", c=DCH))
    wk_sb = wx.tile([P, DCH, JC], BF16)
    nc.sync.dma_start(wk_sb[:], t["wk"].rearrange("p (c n) -> p c n", c=DCH))
    wq_sb = wx.tile([P, DCH, JC], BF16)
    nc.sync.dma_start(wq_sb[:], t["wq"].rearrange("p (c n) -> p c n", c=DCH))
    wkv_sb = wx.tile([P, DCH, JC], BF16)

    x_cm = tc.tile_pool(name="x", bufs=1)
    xp = x_cm.__enter__()
    kt_sb = xp.tile([P, DCH, S], BF16)
    for dc in range(DCH):
        nc.sync.dma_start(
            kt_sb[:, dc, :],
            t["kt"].rearrange("p (c n) -> p c n", c=DCH)[:, dc, :])
    qt_sb = xp.tile([P, DCH, S], BF16)
    for dc in range(DCH):
        nc.sync.dma_start(
            qt_sb[:, dc, :],
            t["qt"].rearrange("p (c n) -> p c n", c=DCH)[:, dc, :])
    wo_sb = const.tile([P, DCH, D], BF16)
    nc.sync.dma_start(wo_sb[:], t["wo"].rearrange("p (c n) -> p c n", c=DCH))

    # ---- Wkv = Wk @ Wv (local JC columns); vp bias = bk @ Wv + bv ----
    for dc in range(DCH):
        psW = ps_s.tile([P, JC], F32, tag="ps_s")
        for ec in range(DCH):
            nc.tensor.matmul(psW[:], wkt_sb[:, ec, dc * P:(dc + 1) * P],
                             wv_sb[:, ec, :], start=(ec == 0), stop=(ec == DCH - 1))
        nc.vector.tensor_copy(wkv_sb[:, dc, :], psW[:])
    psB = ps_s.tile([1, JC], F32, tag="ps_s")
    for ec in range(DCH):
        nc.tensor.matmul(psB[:], bkc_sb[:, ec:ec + 1], wv_sb[:, ec, :],
                         start=(ec == 0), stop=(ec == DCH - 1))
    vpb_row = const.tile([1, JC], F32)
    nc.vector.tensor_tensor(out=vpb_row[:], in0=psB[:], in1=bv_sb[:], op=OP.add)
    vpb = const.tile([P, JC], F32)
    nc.gpsimd.partition_broadcast(vpb[:], vpb_row[:], channels=P)

    # ---- Kp^T / Qp^T projections: heads[j, s] = sum_d W[d, j] X^T[d, s] ----
    # Bias adds split between the Act engine (K path) and the DVE (Q path)
    # so the prolog is not DVE-serialized.
    def project(w_sb, x_sb, bias_sb, dst):
        for jc2 in range(JC // P):          # 2 head-pairs
            for nb in range(S // QB):       # 4 s-blocks
                ps = ps_s.tile([P, QB], F32, tag="ps_s")
                for dc in range(DCH):
                    nc.tensor.matmul(
                        ps[:], w_sb[:, dc, jc2 * P:(jc2 + 1) * P],
                        x_sb[:, dc, nb * QB:(nb + 1) * QB],
                        start=(dc == 0), stop=(dc == DCH - 1))
                for hh in range(2):
                    h = 2 * jc2 + hh
                    nc.vector.tensor_scalar(
                        out=dst[:, h, nb * QB:(nb + 1) * QB],
                        in0=ps[hh * DH:(hh + 1) * DH, :],
                        scalar1=bias_sb[:, h:h + 1], scalar2=None,
                        op0=OP.add)
    project(wk_sb, kt_sb, bk_sb, k_heads)

    # ---- Vp natural [s, j] = sum_d K^T[d, s]^T Wkv[d, j], + bias, fp8 ----
    for sc in range(SCH):
        psV = ps_s.tile([P, JC], F32, tag="ps_s")
        for dc in range(DCH):
            nc.tensor.matmul(psV[:], kt_sb[:, dc, sc * P:(sc + 1) * P],
                             wkv_sb[:, dc, :], start=(dc == 0), stop=(dc == DCH - 1))
        g2, i2 = divmod(sc, 2)
        nc.vector.tensor_tensor(
            out=vp[:, g2, :, i2, 0:DH],
            in0=psV.rearrange("p (h d) -> p h d", h=HL),
            in1=vpb.rearrange("p (h d) -> p h d", h=HL), op=OP.add)
    nc.vector.memset(vp[:, :, :, :, DH:DH + 1], 1.0)
    nc.vector.memset(vp[:, :, :, :, DH + 1:DH + 16], 0.0)

    project(wq_sb, qt_sb, bq_sb, q_heads)

    x_cm.__exit__(None, None, None)        # free kt/qt (64 KB/part)
    wx_cm.__exit__(None, None, None)       # free weights (32 KB/part)

    # ---- attention: software-pipelined chunk stream ----
    # Chunks (unit, g) run in a flat stream; attn@V lags SKEW chunks behind
    # the scores so the in-order PE queue never blocks on an exp still in
    # flight. Each unit's normalize/transpose tail is deferred into the next
    # unit's stream slots the same way.
    from collections import deque

    epool_cm = tc.tile_pool(name="epool", bufs=7)
    epool = epool_cm.__enter__()
    ipool_cm = tc.tile_pool(name="ipool", bufs=4)
    ipool = ipool_cm.__enter__()
    opool_cm = tc.tile_pool(name="opool", bufs=2)
    opool = opool_cm.__enter__()

    # chunked AllToAll: the host permutes Q's sequence dim so q-block qb
    # holds exactly the rows destined to row-slice qb of every core; each
    # 256KB collective fires as soon as its q-block's attention completes
    # and overlaps the remaining attention.
    a2a_in = [dram.tile([QB, JC], BF16, name=f"a2a_in{i}") for i in range(NQB)]
    a2a_out = [dram.tile([QB, JC], BF16, name=f"a2a_out{i}")
               for i in range(NQB)]

    NG = SCH // 2                           # 8 kc-pair groups
    UNITS = [(qb, h) for qb in range(NQB) for h in range(HL)]
    NCH = len(UNITS) * NG
    SKEW = 5

    e_tiles = {}
    psA_tiles = {}
    pending = deque()

    def emit_scores(c):
        u, g = divmod(c, NG)
        qb, h = UNITS[u]
        qsl = slice(qb * QB, (qb + 1) * QB)
        psS = ps_s.tile([P, 2 * QB], F32, tag="ps_s")
        for i in range(2):
            kc = 2 * g + i
            nc.tensor.matmul(
                psS[:, i * QB:(i + 1) * QB],
                k_heads[:, h, kc * P:(kc + 1) * P],
                q_heads[:, h, qsl], start=True, stop=True)
        e_sb = epool.tile([P, 2, QB], FP8, tag="e")
        if g >= DVE_CHUNKS[u % 2]:
            nc.scalar.activation(
                e_sb.rearrange("p a b -> p (a b)"), psS[:], AF.Exp,
                scale=0.125, bias=smshift_t[:])
        else:
            i32_sb = ipool.tile([P, 2 * QB], I32, tag="i32")
            nc.vector.tensor_scalar(
                out=i32_sb[:], in0=psS[:], scalar1=EXA * 0.125,
                scalar2=EXB - SM_SHIFT * EXA, op0=OP.mult, op1=OP.add)
            nc.vector.tensor_copy(
                e_sb.rearrange("p a b -> p (a b)"), i32_sb.bitcast(F32))
        e_tiles[c] = e_sb

    def emit_pv(c):
        u, g = divmod(c, NG)
        qb, h = UNITS[u]
        if g == 0:
            psA_tiles[u] = ps_a.tile([DH + 16, QB], F32, tag="psA",
                                     name="psA")
        nc.tensor.matmul(
            psA_tiles[u][:], vp[:, g, h, :, :], e_tiles.pop(c)[:],
            start=(g == 0), stop=(g == NG - 1), perf_mode=DR)
        if g == NG - 1:
            pending.extend(_post_pieces(u))

    def _post_pieces(u):
        qb, h = UNITS[u]
        psA = psA_tiles.pop(u)
        st = {}

        def p_copy():
            # copy on the Act engine: its queue drains faster than the DVE's
            # mid-attention, so the PE transposes waiting on oht stall less.
            st["oht"] = opool.tile([DH + 1, QB], BF16, name="oht")
            nc.scalar.activation(st["oht"][:], psA[0:DH + 1, :], AF.Copy)

        def p_tr(qc0):
            def f():
                if "pst" not in st:
                    st["pst"] = ps_a.tile([P, NQB, 2 * P], BF16,
                                          tag="pst", name="pst")
                oht, pst = st["oht"], st["pst"]
                for qc in (qc0, qc0 + 1):
                    nc.tensor.transpose(
                        pst[:, qc, 0:DH + 1],
                        oht[:, qc * P:(qc + 1) * P],
                        ident_b[0:DH + 1, 0:DH + 1])
                    nc.tensor.transpose(
                        pst[:, qc, DH + 2:2 * DH + 2],
                        q_heads[:, h,
                                (qb * NQB + qc) * P:(qb * NQB + qc + 1) * P],
                        ident_b[0:DH, 0:DH])
            return f

        def p_norm():
            pst = st["pst"]
            rec = opool.tile([P, NQB, 1], F32, name="rec")
            nc.vector.reciprocal(rec[:], pst[:, :, DH:DH + 1])
            for qc in range(NQB):
                sc = qb * NQB + qc
                nc.vector.tensor_scalar(
                    out=oh_nat[:, sc, h * DH:(h + 1) * DH],
                    in0=pst[:, qc, 0:DH], scalar1=rec[:, qc, :],
                    scalar2=None, op0=OP.mult)

        def p_res():
            pst = st["pst"]
            nc.vector.tensor_tensor(
                out=oh_nat[:, qb * NQB:(qb + 1) * NQB, h * DH:(h + 1) * DH],
                in0=oh_nat[:, qb * NQB:(qb + 1) * NQB, h * DH:(h + 1) * DH],
                in1=pst[:, :, DH + 2:2 * DH + 2], op=OP.add)

        pieces = [p_copy, None, None, p_tr(0), p_tr(2), p_norm, p_res]
        if h == HL - 1:
            def p_dma():
                for qc in range(NQB):
                    sc = qb * NQB + qc
                    nc.sync.dma_start(a2a_in[qb][qc * P:(qc + 1) * P, :],
                                      oh_nat[:, sc, :])
                nc.gpsimd.collective_compute(
                    "AllToAll", OP.bypass, ins=[a2a_in[qb].opt()],
                    outs=[a2a_out[qb].opt()],
                    replica_groups=[list(range(NCORES))])
            pieces.append(p_dma)
        return pieces

    for c in range(NCH + SKEW):
        if c >= SKEW:
            emit_pv(c - SKEW)
        if c < NCH:
            emit_scores(c)
        if pending:
            piece = pending.popleft()
            if piece is not None:
                piece()
    while pending:
        piece = pending.popleft()
        if piece is not None:
            piece()

    opool_cm.__exit__(None, None, None)
    ipool_cm.__exit__(None, None, None)
    epool_cm.__exit__(None, None, None)
    heads_cm.__exit__(None, None, None)

    # ---- stage 2: rows [SR2, D] : LN0 -> Wo+gelu+residual -> LN1 ----
    s2 = ctx.enter_context(tc.tile_pool(name="s2", bufs=1))
    ln_tmp = ctx.enter_context(tc.tile_pool(name="ln_tmp", bufs=4))
    NS2 = SR2 // P                                    # 4 row chunks
    # chunked-A2A gather: a2a_out[qb] rows [64c:64c+64) hold, for peer c,
    # this core's local rows 256*(c//4) + 64*qb + [0,64) (batch c//4),
    # columns [256*(c%4):...). Local row r -> o_sb[r%128, r//128, :].
    o_sb = s2.tile([P, NS2, D], BF16)
    for qb in range(NQB):
        for c in range(NCORES):
            bb, jj = divmod(c, GROUP)
            rloc = 256 * bb + 64 * qb
            sc2, rr = divmod(rloc, P)
            nc.sync.dma_start(
                o_sb[rr:rr + 64, sc2, jj * JC:(jj + 1) * JC],
                a2a_out[qb][c * 64:(c + 1) * 64, :])

    def layernorm(src_ap, dst_ap, gb, bb):
        """src [128, D] -> dst [128, D] layernorm with broadcast gamma/beta."""
        red = ln_tmp.tile([P, 1], F32, tag="red")
        nc.vector.tensor_reduce(red[:], src_ap, mybir.AxisListType.X, OP.add)
        negmean = ln_tmp.tile([P, 1], F32, tag="negmean")
        nc.vector.tensor_scalar_mul(negmean[:], red[:], -1.0 / D)
        sq = ln_tmp.tile([P, D], BF16, tag="sq")
        sumsq = ln_tmp.tile([P, 1], F32, tag="sumsq")
        nc.scalar.activation(sq[:], src_ap, AF.Square, bias=negmean[:],
                             scale=1.0, accum_out=sumsq[:])
        std = ln_tmp.tile([P, 1], F32, tag="std")
        nc.scalar.activation(std[:], sumsq[:], AF.Sqrt, bias=eps_t[:],
                             scale=1.0 / D)
        rstd = ln_tmp.tile([P, 1], F32, tag="rstd")
        nc.vector.reciprocal(rstd[:], std[:])
        nc.vector.tensor_scalar(out=dst_ap, in0=src_ap, scalar1=negmean[:],
                                scalar2=rstd[:], op0=OP.add, op1=OP.mult)
        nc.vector.tensor_tensor(out=dst_ap, in0=dst_ap, in1=gb[:], op=OP.mult)
        nc.vector.tensor_tensor(out=dst_ap, in0=dst_ap, in1=bb[:], op=OP.add)

    # sc2 chunks 0 and 2 only need the first two A2A chunks - process them
    # first so LN0 overlaps the tail of attention.
    SC2_ORDER = (0, 2, 1, 3)
    ln0 = s2.tile([P, NS2, D], BF16)
    for sc2 in SC2_ORDER:
        layernorm(o_sb[:, sc2, :], ln0[:, sc2, :], bcast["g0"], bcast["b0"])

    # transpose ln0 -> [128, DCH, SR2] for the Wo contraction
    ln0t = s2.tile([P, DCH, SR2], BF16)
    for dc in range(DCH):
        psL = ps_a.tile([P, SR2], BF16, tag="pst")
        for sc2 in range(NS2):
            nc.tensor.transpose(psL[:, sc2 * P:(sc2 + 1) * P],
                                ln0[:, sc2, dc * P:(dc + 1) * P], ident_b)
        nc.vector.tensor_copy(ln0t[:, dc, :], psL[:])

    o2 = s2.tile([P, NS2, D], BF16)
    for sc2 in SC2_ORDER:
        psF = ps_s.tile([P, D], F32, tag="ps_s")
        for dc in range(DCH):
            for nb in range(2):
                nc.tensor.matmul(
                    psF[:, nb * QB:(nb + 1) * QB],
                    ln0t[:, dc, sc2 * P:(sc2 + 1) * P],
                    wo_sb[:, dc, nb * QB:(nb + 1) * QB],
                    start=(dc == 0), stop=(dc == DCH - 1))
        fb = ln_tmp.tile([P, D], F32, tag="fb")
        nc.vector.tensor_tensor(out=fb[:], in0=psF[:], in1=bcast["bo"][:],
                                op=OP.add)
        gel = ln_tmp.tile([P, D], BF16, tag="gel")
        nc.scalar.activation(gel[:], fb[:], AF.Gelu)
        nc.vector.tensor_tensor(out=o2[:, sc2, :], in0=ln0[:, sc2, :],
                                in1=gel[:], op=OP.add)

    for sc2 in SC2_ORDER:
        fin = ln_tmp.tile([P, D], F32, tag="fin")
        layernorm(o2[:, sc2, :], fin[:], bcast["g1"], bcast["b1"])
        nc.sync.dma_start(t["out"][sc2 * P:(sc2 + 1) * P, :], fin[:])


def build():
    if "nc" in _CACHE:
        return _CACHE["nc"]
    from contextlib import ExitStack
    nc = bacc.Bacc("TRN2", target_bir_lowering=False, debug=False,
                   num_devices=NCORES)
    t = _declare_io(nc)
    with tile.TileContext(nc) as tc:
        with ExitStack() as ctx:
            _emit(nc, tc, ctx, t)
    nc.compile()
    _CACHE["nc"] = nc
    return nc


def _pmajor(a):
    """[D0, N] with D0 = c*128+p  ->  [128, c*N] partition-major bf16."""
    d0, n = a.shape
    c = d0 // P
    return np.ascontiguousarray(
        a.reshape(c, P, n).transpose(1, 0, 2).reshape(P, c * n).astype(NPBF16))


def make_in_maps(Q, K, Wq, bq, Wk, bk, Wv, bv, Wo, bo, g0, b0, g1, b1):
    f32 = np.float32
    in_maps = []
    wkt_full = _pmajor(np.ascontiguousarray(Wk.T))
    wo_full = _pmajor(Wo)
    bkc = np.ascontiguousarray(
        bk.reshape(DCH, P).T.astype(NPBF16))          # [128, DCH]
    qt = {}
    kt = {}
    for b in range(2):
        qt[b] = _pmajor(np.ascontiguousarray(Q[b].T[:, _PERM]))
        kt[b] = _pmajor(np.ascontiguousarray(K[b].T))
    for c in range(NCORES):
        b, g = divmod(c, GROUP)
        jsl = slice(g * JC, (g + 1) * JC)
        in_maps.append({
            "qt": qt[b], "kt": kt[b],
            "wq": _pmajor(Wq[:, jsl]), "wk": _pmajor(Wk[:, jsl]),
            "wv": _pmajor(Wv[:, jsl]),
            "wkt": wkt_full, "wo": wo_full,
            "bq_h": np.ascontiguousarray(
                bq[jsl].reshape(HL, DH).T.astype(f32)),
            "bk_h": np.ascontiguousarray(
                bk[jsl].reshape(HL, DH).T.astype(f32)),
            "bkc": bkc,
            "bv_s": np.ascontiguousarray(bv[jsl].reshape(1, JC).astype(f32)),
            "bo": np.ascontiguousarray(bo.reshape(1, D).astype(f32)),
            "g0": np.ascontiguousarray(g0.reshape(1, D).astype(f32)),
            "b0": np.ascontiguousarray(b0.reshape(1, D).astype(f32)),
            "g1": np.ascontiguousarray(g1.reshape(1, D).astype(f32)),
            "b1": np.ascontiguousarray(b1.reshape(1, D).astype(f32)),
        })
    return in_maps


def run(in_maps, trace=False, **kwargs):
    nc = build()
    return bass_utils.run_bass_kernel_spmd(
        nc, in_maps, core_ids=list(range(NCORES)), trace=trace, **kwargs)


def kernel(**inputs):
    inputs = {k: np.asarray(v, dtype=np.float32) for k, v in inputs.items()}
    in_maps = make_in_maps(
        inputs["Q"], inputs["K"], inputs["Wq"], inputs["bq"], inputs["Wk"],
        inputs["bk"], inputs["Wv"], inputs["bv"], inputs["Wo"], inputs["bo"],
        inputs["g0"], inputs["b0"], inputs["g1"], inputs["b1"])
    res = run(in_maps, trace=False)
    B = 2
    RS = S // NCORES  # 256 rows of each batch per core
    out = np.empty((B, S, D), dtype=np.float32)
    for c in range(NCORES):
        r = res.results[c]["out"]  # [512, D]: rows 0-255 -> b0, 256-511 -> b1
        out[0, c * RS:(c + 1) * RS, :] = r[:RS]
        out[1, c * RS:(c + 1) * RS, :] = r[RS:]
    return out


if __name__ == "__main__":
    rng = np.random.default_rng(0)
    ins = {n: rng.standard_normal(s).astype(np.float32) * (0.03125 if n.startswith("W") else 1.0)
           for n, s in [("Q", (2, S, D)), ("K", (2, S, D)), ("Wq", (D, D)),
                        ("Wk", (D, D)), ("Wv", (D, D)), ("Wo", (D, D))]}
    for n in ("bq", "bk", "bv", "bo", "b0", "b1"):
        ins[n] = np.zeros(D, np.float32)
    for n in ("g0", "g1"):
        ins[n] = np.ones(D, np.float32)
    out = kernel(**ins)
    print("ran ok", out.shape, out.dtype)
